# revision 20
# baseline (speedup 1.0000x reference)
"""BAM self-attention block (B=8, C=256, H=W=64) on 8 TRN2 NeuronCores.

Sharding: data-parallel over batch - one batch element per core; the small
1x1-conv weights are replicated to every core.

Per-core algorithm (x is [C=256, N=4096]; all matmuls on the PE, bf16
operands with fp32 PSUM accumulation):
  q = Wq x + bq   [32, N] replicated to 4 PE row groups via column-replicated
                  transposed weights (one matmul writes all 4 replicas)
  k = Wk x + bk   [32, N] likewise
  vT = (Wv x)^T   [N, 256] (bias bv folded into the output residual, since
                  softmax rows sum to 1)
  S^T[n, m] = sum_c k[c,n] q[c,m]  computed directly transposed so the second
              matmul's contraction (over n) lies on partitions; 4 key-blocks
              run concurrently via PE row-tiling (K=32 each), split across
              TWO 2-bank PSUM tiles (stA: j=0,1; stB: j=2,3).
  P^T = exp(S^T)  TWO half-tile ACT passes (expA/expB) -> bf16, so the next
                  group's S^T j=0,1 can overwrite stA while expB still runs:
                  the serial exp->S^T->exp chain of the single-tile version
                  is broken and ACT+PE stay co-saturated.
  s[m] = sum_n P^T[n, m]  4 col-tiled M=1 ones-matmuls (concurrent) + a
                  4x K=1 row-tiled reduce+broadcast matmul that overwrites
                  s_ps in place (no DMA gather), then a fast DVE reciprocal
  out[c, m] = sum_n vT[n, c] P^T[n, m]  accumulated in PSUM over all 32 blocks
  y = gamma/s * out + (x + gamma*bv)

Emission order per group: [S^T-A(next), out-A(cur), S^T-B(next), out-B(cur),
ssum(cur)] so each half of the next group's S^T is issued as soon as the
matching half-exp of the current group frees its PSUM banks.
"""
import sys
import numpy as np

for p in ("/opt/trn_rl_repo",):
    if p not in sys.path:
        sys.path.insert(0, p)

B, C, H, W = 8, 256, 64, 64
N = H * W          # 4096
CK = C // 8        # 32
NB = N // 128      # 32 key blocks
MC = N // 512      # 8 query chunks
NG = NB // 4       # 8 groups of 4 key blocks

_NC_CACHE = {}


def _build_nc():
    import concourse.mybir as mybir
    import concourse.tile as tile
    from concourse import bacc
    from concourse.bass import ds

    f32, f32r, bf16 = mybir.dt.float32, mybir.dt.float32r, mybir.dt.bfloat16
    Exp = mybir.ActivationFunctionType.Exp
    Identity = mybir.ActivationFunctionType.Identity

    nc = bacc.Bacc("TRN2", target_bir_lowering=False, debug=False)

    # x is declared float32r (same bits as fp32): the projection matmuls
    # then stream it directly in single-pass fp32 mode at 1 col/cycle
    x_d = nc.dram_tensor("x", [C, N], f32r, kind="ExternalInput").ap()
    wq_d = nc.dram_tensor("Wq", [CK, C], f32, kind="ExternalInput").ap()
    bq_d = nc.dram_tensor("bq", [CK], f32, kind="ExternalInput").ap()
    wk_d = nc.dram_tensor("Wk", [CK, C], f32, kind="ExternalInput").ap()
    bk_d = nc.dram_tensor("bk", [CK], f32, kind="ExternalInput").ap()
    wv_d = nc.dram_tensor("Wv", [C, C], f32, kind="ExternalInput").ap()
    bv_d = nc.dram_tensor("bv", [C], f32, kind="ExternalInput").ap()
    g_d = nc.dram_tensor("gamma", [1], f32, kind="ExternalInput").ap()
    y_d = nc.dram_tensor("y", [C, N], f32, kind="ExternalOutput").ap()

    x_r = x_d.rearrange("(o p) n -> p o n", p=128)   # c = o*128 + p
    y_r = y_d.rearrange("(o p) n -> p o n", p=128)

    with tile.TileContext(nc) as tc:
        with tc.tile_pool(name="const", bufs=1) as const, \
             tc.tile_pool(name="big", bufs=1) as big, \
             tc.tile_pool(name="work", bufs=4) as work, \
             tc.tile_pool(name="ptp", bufs=8) as ptp, \
             tc.tile_pool(name="ps_st", bufs=1, space="PSUM") as ps_st, \
             tc.tile_pool(name="ps_out", bufs=2, space="PSUM") as ps_out, \
             tc.tile_pool(name="ps_misc", bufs=1, space="PSUM") as ps_misc:

            # ---------- constants / weights (natural layout, transposed on PE)
            from concourse.masks import make_identity
            ident = const.tile([128, 128], f32, tag="ident")
            make_identity(nc, ident[:])

            # biases: bq/bk replicated to all 4 row groups
            bq4 = const.tile([128, 1], f32, tag="bq4")
            bk4 = const.tile([128, 1], f32, tag="bk4")
            for j in range(4):
                nc.gpsimd.dma_start(bq4[32 * j:32 * (j + 1), :], bq_d[:, None])
                nc.gpsimd.dma_start(bk4[32 * j:32 * (j + 1), :], bk_d[:, None])
            bv2 = const.tile([128, 2], f32, tag="bv2")
            nc.gpsimd.dma_start(bv2[:], bv_d.rearrange("(o p) -> p o", p=128))
            g_col = const.tile([128, 1], f32, tag="gcol")
            nc.gpsimd.dma_start(g_col[:], g_d[None, :].to_broadcast([128, 1]))

            ones1 = const.tile([128, 1], bf16, tag="ones1")
            nc.any.memset(ones1[:], 1.0)
            ones4_raw = work.tile([4, 128], f32, tag="o4raw")
            nc.any.memset(ones4_raw[:], 1.0)
            ones4 = const.tile([4, 128], f32r, tag="ones4")
            nc.vector.tensor_copy(ones4[:], ones4_raw[:])

            gbv = const.tile([128, 2], f32, tag="gbv")
            nc.vector.tensor_scalar_mul(gbv[:], bv2[:], g_col[:])

            # Wq/Wk [32, 256] natural -> transpose chunks -> wqT/wkT [128, 2, 32]
            wq_nat = work.tile([CK, C], f32, tag="wnat")
            nc.sync.dma_start(wq_nat[:], wq_d[:])
            wk_nat = work.tile([CK, C], f32, tag="wnat")
            nc.sync.dma_start(wk_nat[:], wk_d[:])

            # PE warmup: ~20 dummy matmuls issued while the weight/x DMAs
            # stream, so the HAM clock gate reaches K=8/8 (2.4 GHz) before
            # real work starts and never re-throttles during the sparse
            # projection phase
            warm_sb = work.tile([128, 512], bf16, tag="warm")
            nc.any.memset(warm_sb[:], 0.0)
            warm_ps = ps_st.tile([128, 2048], f32, tag="st", name="warm_ps")
            for _ in range(8):
                nc.tensor.matmul(warm_ps[:, 0:512], warm_sb[:, 0:128],
                                 warm_sb[:], start=True, stop=True)
            # wqT4/wkT4: transposed weights with the 32 columns replicated 4x,
            # so one matmul yields q replicated across all 4 PE row groups
            wqT4 = const.tile([128, 2, 128], f32r, tag="wqT4")
            wkT4 = const.tile([128, 2, 128], f32r, tag="wkT4")
            for nat, dstw, ptag in ((wq_nat, wqT4, "sacc"),
                                    (wk_nat, wkT4, "srep")):
                for o in range(2):
                    tp = ps_misc.tile([128, 512], f32, tag=ptag)
                    nc.tensor.transpose(tp[:, 0:CK], nat[:, ds(128 * o, 128)],
                                        ident[0:CK, 0:CK])
                    for j in range(4):
                        nc.vector.tensor_copy(dstw[:, o, ds(32 * j, 32)],
                                              tp[:, 0:CK])

            # Wv [256, 256] natural -> 4 transposed blocks -> wvT [128, 2, 256]
            wv_nat = work.tile([128, 2, C], f32, tag="wvnat")
            wv_n = wv_d.rearrange("(o p) c -> p o c", p=128)
            for o in range(2):
                nc.sync.dma_start(wv_nat[:, o], wv_n[:, o])
            wvT = const.tile([128, 2, C], f32r, tag="wvT")
            for o_c in range(2):
                for o_co in range(2):
                    tp = ps_out.tile([128, 128], f32, tag="out")
                    nc.tensor.transpose(tp[:], wv_nat[:, o_co, ds(128 * o_c, 128)],
                                        ident[:])
                    nc.vector.tensor_copy(wvT[:, o_c, ds(128 * o_co, 128)], tp[:])

            # ---------- x load (after the small weight DMAs, so they are
            # not queued behind 4 MB of x), then cast + projections ----------
            # chunks alternate sync/gpsimd queues: two rings stream in
            # parallel so projections are never DMA-starved
            # xs is float32r: the projection matmuls stream it directly
            # (single-pass fp32 runs at 1 col/cycle like bf16), so no bf16
            # cast of x is needed at all
            xs = big.tile([128, 2, N], f32r, tag="xs")
            for mc in range(MC):
                ms = ds(512 * mc, 512)
                eng = nc.sync if mc % 2 == 0 else nc.scalar
                eng.dma_start(xs[:, :, ms], x_r[:, :, ms])

            q4c = [big.tile([128, 512], bf16, tag=f"q4_{i}", name=f"q4_{i}")
                   for i in range(MC)]
            k4c = [big.tile([128, 512], bf16, tag=f"k4_{i}", name=f"k4_{i}")
                   for i in range(MC)]
            vTc = [big.tile([128, 4, C], bf16, tag=f"vT_{i}", name=f"vT_{i}")
                   for i in range(MC)]
            def st_group(mc, g):
                """Emit the 4 row-tiled S^T matmuls + whole-tile exp."""
                st = ps_st.tile([128, 2048], f32, tag="st", name=f"st_{mc}_{g}")
                for j in range(4):
                    nb = 4 * g + j
                    nc.tensor.matmul(st[:, ds(512 * j, 512)],
                                     k4c[nb // 4][32 * j:32 * (j + 1),
                                                  ds(128 * (nb % 4), 128)],
                                     q4c[mc][32 * j:32 * (j + 1), :],
                                     start=True, stop=True,
                                     tile_position=(32 * j, 0))
                pt = ptp.tile([128, 2048], bf16, tag="pt", name=f"pt_{mc}_{g}")
                nc.scalar.activation(pt[:], st[:], Exp)
                return pt

            # chunk-0 S^T+exp groups are emitted interleaved with the
            # projection chunks they depend on, so the exp pipeline fills
            # during the projection phase's chain-wait bubbles instead of
            # queueing behind all projection matmuls
            PIPE = 6
            ptq = []
            for mc in range(MC):
                ms = ds(512 * mc, 512)
                # q/k: replicated-column weights yield all 4 replicas at
                # once. Their PSUM tiles borrow the sacc/srep banks (idle
                # until attention), so the projection phase runs three
                # independent alloc->read chains instead of one long one.
                for w_t, b4, dst, ptag in ((wqT4, bq4, q4c[mc], "sacc"),
                                           (wkT4, bk4, k4c[mc], "srep")):
                    pp = ps_misc.tile([128, 512], f32, tag=ptag)
                    for o in range(2):
                        nc.tensor.matmul(pp[:], w_t[:, o, :], xs[:, o, ms],
                                         start=(o == 0), stop=(o == 1))
                    nc.scalar.activation(dst[:], pp[:], Identity, bias=b4[:])
                # vT for the 4 key-blocks in this chunk; two blocks share
                # one PSUM tile so the proj phase only cycles the ring twice
                # per chunk (halves the alloc->copy chain that paces it)
                for pair in range(2):
                    pv = ps_out.tile([128, 2, C], f32, tag="out")
                    for bb in range(2):
                        nb = 4 * mc + 2 * pair + bb
                        for o in range(2):
                            nc.tensor.matmul(pv[:, bb, :],
                                             xs[:, o, ds(128 * nb, 128)],
                                             wvT[:, o, :],
                                             start=(o == 0), stop=(o == 1))
                    nc.vector.tensor_copy(
                        vTc[mc][:, ds(2 * pair, 2), :], pv[:])
                # residual base for this chunk: xs += gamma*bv
                for o in range(2):
                    nc.vector.tensor_scalar_add(xs[:, o, ms], xs[:, o, ms],
                                                gbv[:, o:o + 1])
                if mc < PIPE:
                    ptq.append(st_group(0, mc))

            # ---------- main attention loop over query chunks ----------
            # Per group of 4 key-blocks: 4 row-tiled S^T matmuls into one
            # 4-bank PSUM tile, one whole-tile exp on ACT, then (pipelined)
            # 4 adjacent col-tiled s-sums + 8 out accumulations. S^T of group
            # g+1 is emitted before the out-block of g so the PE never waits
            # on ACT in steady state.
            pending_tail = None
            for mc in range(MC):
                ms = ds(512 * mc, 512)
                out_ps = [ps_out.tile([128, 512], f32, tag="out",
                                      name=f"out_{mc}_{cc}")
                          for cc in range(2)]
                s_ps = ps_misc.tile([128, 512], f32, tag="sacc",
                                    name=f"sacc_{mc}")
                for ng in range(NG):
                    gn = mc * NG + ng + PIPE
                    if gn < MC * NG:
                        ptq.append(st_group(gn // NG, gn % NG))
                    pt = ptq.pop(0)
                    for j in range(4):
                        nb = 4 * ng + j
                        for cc in range(2):
                            nc.tensor.matmul(out_ps[cc][:],
                                             vTc[nb // 4][:, nb % 4,
                                                          ds(128 * cc, 128)],
                                             pt[:, ds(512 * j, 512)],
                                             start=(ng == 0 and j == 0),
                                             stop=(ng == NG - 1 and j == 3))
                    # 4 col-tiled partition-sum matmuls, back-to-back
                    for j in range(4):
                        nc.tensor.matmul(s_ps[32 * j:32 * j + 1, :], ones1[:],
                                         pt[:, ds(512 * j, 512)],
                                         start=(ng == 0), stop=(ng == NG - 1),
                                         tile_position=(0, 32 * j))
                    # previous chunk's normalize/output tail: emitted one
                    # group in, so its srep matmul never head-of-line blocks
                    # the queue (its s4 gather DMA is long since done)
                    if ng == 1 and pending_tail is not None:
                        pending_tail()
                        pending_tail = None
                # free the out banks right away; finals run from SBUF copies
                out_sb = []
                for cc in range(2):
                    ob = work.tile([128, 512], f32, tag=f"ob{cc}",
                                   name=f"ob_{mc}_{cc}")
                    nc.vector.tensor_copy(ob[:], out_ps[cc][:])
                    out_sb.append(ob)
                s4c = work.tile([128, 512], f32r, tag="s4c", name=f"s4c_{mc}")
                nc.vector.tensor_copy(s4c[:], s_ps[:])
                # gather the 4 partial rows now (gpsimd queue), so the tail's
                # srep LDWEIGHTS never waits on this DMA at the queue head
                s4_sb = work.tile([4, 512], f32r, tag="s4", name=f"s4_{mc}")
                nc.gpsimd.dma_start(s4_sb[:], s4c[0:97:32, :])

                def tail(mc=mc, ms=ms, out_sb=out_sb, s4_sb=s4_sb):
                    # s: reduce the 4 partial rows + broadcast to 128 rows
                    srep_ps = ps_misc.tile([128, 512], f32, tag="srep")
                    nc.tensor.matmul(srep_ps[:], ones4[:], s4_sb[:],
                                     start=True, stop=True)
                    r_rep = work.tile([128, 512], f32, tag="rrep")
                    nc.vector.reciprocal_approx_fast(r_rep[:], srep_ps[:])
                    nc.vector.tensor_scalar_mul(r_rep[:], r_rep[:], g_col[:])
                    for cc in range(2):
                        y_sb = work.tile([128, 512], f32, tag="y")
                        for h in range(2):
                            hs = ds(256 * h, 256)
                            ys = ds(512 * mc + 256 * h, 256)
                            t_sb = work.tile([128, 256], f32, tag="t")
                            nc.vector.tensor_mul(t_sb[:], out_sb[cc][:, hs],
                                                 r_rep[:, hs])
                            nc.vector.tensor_add(y_sb[:, hs], t_sb[:],
                                                 xs[:, cc, ys])
                            nc.sync.dma_start(y_r[:, cc, ys], y_sb[:, hs])

                pending_tail = tail
            pending_tail()

    nc.compile()
    return nc


def kernel(x, Wq, bq, Wk, bk, Wv, bv, gamma):
    from concourse import bass_utils

    if "nc" not in _NC_CACHE:
        _NC_CACHE["nc"] = _build_nc()
    nc = _NC_CACHE["nc"]

    x = np.ascontiguousarray(np.asarray(x, dtype=np.float32))
    shared = {
        "Wq": np.ascontiguousarray(np.asarray(Wq, dtype=np.float32)),
        "bq": np.ascontiguousarray(np.asarray(bq, dtype=np.float32)),
        "Wk": np.ascontiguousarray(np.asarray(Wk, dtype=np.float32)),
        "bk": np.ascontiguousarray(np.asarray(bk, dtype=np.float32)),
        "Wv": np.ascontiguousarray(np.asarray(Wv, dtype=np.float32)),
        "bv": np.ascontiguousarray(np.asarray(bv, dtype=np.float32)),
        "gamma": np.ascontiguousarray(np.asarray(gamma, dtype=np.float32)),
    }
    in_maps = [dict(shared, x=np.ascontiguousarray(x[i].reshape(C, N)))
               for i in range(B)]

    res = bass_utils.run_bass_kernel_spmd(nc, in_maps, core_ids=list(range(B)))
    y = np.stack([res.results[i]["y"] for i in range(B)], axis=0)
    return y.reshape(B, C, H, W).astype(np.float32)


if __name__ == "__main__":
    rng = np.random.default_rng(0)
    ins = {
        "x": rng.standard_normal((B, C, H, W), dtype=np.float32),
        "Wq": rng.standard_normal((CK, C), dtype=np.float32) / 16,
        "bq": rng.standard_normal((CK,), dtype=np.float32) * 0.01,
        "Wk": rng.standard_normal((CK, C), dtype=np.float32) / 16,
        "bk": rng.standard_normal((CK,), dtype=np.float32) * 0.01,
        "Wv": rng.standard_normal((C, C), dtype=np.float32) / 16,
        "bv": rng.standard_normal((C,), dtype=np.float32) * 0.01,
        "gamma": rng.standard_normal((1,), dtype=np.float32) * 0.1,
    }
    y = kernel(**ins)
    print("kernel output", y.shape, y.dtype)


# revision 21
# speedup vs baseline: 1.0017x; 1.0017x over previous
"""BAM self-attention block (B=8, C=256, H=W=64) on 8 TRN2 NeuronCores.

Sharding: data-parallel over batch - one batch element per core; the small
1x1-conv weights are replicated to every core.

Per-core algorithm (x is [C=256, N=4096]; all matmuls on the PE, bf16
operands with fp32 PSUM accumulation):
  q = Wq x + bq   [32, N] replicated to 4 PE row groups via column-replicated
                  transposed weights (one matmul writes all 4 replicas)
  k = Wk x + bk   [32, N] likewise
  vT = (Wv x)^T   [N, 256] (bias bv folded into the output residual, since
                  softmax rows sum to 1)
  S^T[n, m] = sum_c k[c,n] q[c,m]  computed directly transposed so the second
              matmul's contraction (over n) lies on partitions; 4 key-blocks
              run concurrently via PE row-tiling (K=32 each), split across
              TWO 2-bank PSUM tiles (stA: j=0,1; stB: j=2,3).
  P^T = exp(S^T)  TWO half-tile ACT passes (expA/expB) -> bf16, so the next
                  group's S^T j=0,1 can overwrite stA while expB still runs:
                  the serial exp->S^T->exp chain of the single-tile version
                  is broken and ACT+PE stay co-saturated.
  s[m] = sum_n P^T[n, m]  4 col-tiled M=1 ones-matmuls (concurrent) + a
                  4x K=1 row-tiled reduce+broadcast matmul that overwrites
                  s_ps in place (no DMA gather), then a fast DVE reciprocal
  out[c, m] = sum_n vT[n, c] P^T[n, m]  accumulated in PSUM over all 32 blocks
  y = gamma/s * out + (x + gamma*bv)

Emission order per group: [S^T-A(next), out-A(cur), S^T-B(next), out-B(cur),
ssum(cur)] so each half of the next group's S^T is issued as soon as the
matching half-exp of the current group frees its PSUM banks.
"""
import sys
import numpy as np

for p in ("/opt/trn_rl_repo",):
    if p not in sys.path:
        sys.path.insert(0, p)

B, C, H, W = 8, 256, 64, 64
N = H * W          # 4096
CK = C // 8        # 32
NB = N // 128      # 32 key blocks
MC = N // 512      # 8 query chunks
NG = NB // 4       # 8 groups of 4 key blocks

_NC_CACHE = {}


def _build_nc():
    import concourse.mybir as mybir
    import concourse.tile as tile
    from concourse import bacc
    from concourse.bass import ds

    f32, f32r, bf16 = mybir.dt.float32, mybir.dt.float32r, mybir.dt.bfloat16
    Exp = mybir.ActivationFunctionType.Exp
    Identity = mybir.ActivationFunctionType.Identity

    nc = bacc.Bacc("TRN2", target_bir_lowering=False, debug=False)

    # x is declared float32r (same bits as fp32): the projection matmuls
    # then stream it directly in single-pass fp32 mode at 1 col/cycle
    x_d = nc.dram_tensor("x", [C, N], f32r, kind="ExternalInput").ap()
    wq_d = nc.dram_tensor("Wq", [CK, C], f32, kind="ExternalInput").ap()
    bq_d = nc.dram_tensor("bq", [CK], f32, kind="ExternalInput").ap()
    wk_d = nc.dram_tensor("Wk", [CK, C], f32, kind="ExternalInput").ap()
    bk_d = nc.dram_tensor("bk", [CK], f32, kind="ExternalInput").ap()
    wv_d = nc.dram_tensor("Wv", [C, C], f32, kind="ExternalInput").ap()
    bv_d = nc.dram_tensor("bv", [C], f32, kind="ExternalInput").ap()
    g_d = nc.dram_tensor("gamma", [1], f32, kind="ExternalInput").ap()
    y_d = nc.dram_tensor("y", [C, N], f32, kind="ExternalOutput").ap()

    x_r = x_d.rearrange("(o p) n -> p o n", p=128)   # c = o*128 + p
    y_r = y_d.rearrange("(o p) n -> p o n", p=128)

    with tile.TileContext(nc) as tc:
        with tc.tile_pool(name="const", bufs=1) as const, \
             tc.tile_pool(name="big", bufs=1) as big, \
             tc.tile_pool(name="work", bufs=4) as work, \
             tc.tile_pool(name="ptp", bufs=8) as ptp, \
             tc.tile_pool(name="ps_st", bufs=1, space="PSUM") as ps_st, \
             tc.tile_pool(name="ps_out", bufs=2, space="PSUM") as ps_out, \
             tc.tile_pool(name="ps_misc", bufs=1, space="PSUM") as ps_misc:

            # ---------- constants / weights (natural layout, transposed on PE)
            from concourse.masks import make_identity
            ident = const.tile([128, 128], f32, tag="ident")
            make_identity(nc, ident[:])

            # biases: bq/bk replicated to all 4 row groups
            bq4 = const.tile([128, 1], f32, tag="bq4")
            bk4 = const.tile([128, 1], f32, tag="bk4")
            for j in range(4):
                nc.gpsimd.dma_start(bq4[32 * j:32 * (j + 1), :], bq_d[:, None])
                nc.gpsimd.dma_start(bk4[32 * j:32 * (j + 1), :], bk_d[:, None])
            bv2 = const.tile([128, 2], f32, tag="bv2")
            nc.gpsimd.dma_start(bv2[:], bv_d.rearrange("(o p) -> p o", p=128))
            g_col = const.tile([128, 1], f32, tag="gcol")
            nc.gpsimd.dma_start(g_col[:], g_d[None, :].to_broadcast([128, 1]))

            ones1 = const.tile([128, 1], bf16, tag="ones1")
            nc.any.memset(ones1[:], 1.0)
            ones4_raw = work.tile([4, 128], f32, tag="o4raw")
            nc.any.memset(ones4_raw[:], 1.0)
            ones4 = const.tile([4, 128], f32r, tag="ones4")
            nc.vector.tensor_copy(ones4[:], ones4_raw[:])

            gbv = const.tile([128, 2], f32, tag="gbv")
            nc.vector.tensor_scalar_mul(gbv[:], bv2[:], g_col[:])

            # Wq/Wk [32, 256] natural -> transpose chunks -> wqT/wkT [128, 2, 32]
            wq_nat = work.tile([CK, C], f32, tag="wnat")
            nc.sync.dma_start(wq_nat[:], wq_d[:])
            wk_nat = work.tile([CK, C], f32, tag="wnat")
            nc.sync.dma_start(wk_nat[:], wk_d[:])

            # PE warmup: ~20 dummy matmuls issued while the weight/x DMAs
            # stream, so the HAM clock gate reaches K=8/8 (2.4 GHz) before
            # real work starts and never re-throttles during the sparse
            # projection phase
            warm_sb = work.tile([128, 512], bf16, tag="warm")
            nc.any.memset(warm_sb[:], 0.0)
            warm_ps = ps_st.tile([128, 2048], f32, tag="st", name="warm_ps")
            for _ in range(8):
                nc.tensor.matmul(warm_ps[:, 0:512], warm_sb[:, 0:128],
                                 warm_sb[:], start=True, stop=True)
            # wqT4/wkT4: transposed weights with the 32 columns replicated 4x,
            # so one matmul yields q replicated across all 4 PE row groups
            wqT4 = const.tile([128, 2, 128], f32r, tag="wqT4")
            wkT4 = const.tile([128, 2, 128], f32r, tag="wkT4")
            for nat, dstw, ptag in ((wq_nat, wqT4, "sacc"),
                                    (wk_nat, wkT4, "srep")):
                for o in range(2):
                    tp = ps_misc.tile([128, 512], f32, tag=ptag)
                    nc.tensor.transpose(tp[:, 0:CK], nat[:, ds(128 * o, 128)],
                                        ident[0:CK, 0:CK])
                    for j in range(4):
                        nc.vector.tensor_copy(dstw[:, o, ds(32 * j, 32)],
                                              tp[:, 0:CK])

            # Wv [256, 256] natural -> 4 transposed blocks -> wvT [128, 2, 256]
            wv_nat = work.tile([128, 2, C], f32, tag="wvnat")
            wv_n = wv_d.rearrange("(o p) c -> p o c", p=128)
            for o in range(2):
                nc.sync.dma_start(wv_nat[:, o], wv_n[:, o])
            wvT = const.tile([128, 2, C], f32r, tag="wvT")
            for o_c in range(2):
                for o_co in range(2):
                    tp = ps_out.tile([128, 128], f32, tag="out")
                    nc.tensor.transpose(tp[:], wv_nat[:, o_co, ds(128 * o_c, 128)],
                                        ident[:])
                    nc.vector.tensor_copy(wvT[:, o_c, ds(128 * o_co, 128)], tp[:])

            # ---------- x load (after the small weight DMAs, so they are
            # not queued behind 4 MB of x), then cast + projections ----------
            # chunks alternate sync/gpsimd queues: two rings stream in
            # parallel so projections are never DMA-starved
            # xs is float32r: the projection matmuls stream it directly
            # (single-pass fp32 runs at 1 col/cycle like bf16), so no bf16
            # cast of x is needed at all
            xs = big.tile([128, 2, N], f32r, tag="xs")
            for mc in range(MC):
                ms = ds(512 * mc, 512)
                eng = nc.sync if mc % 2 == 0 else nc.scalar
                eng.dma_start(xs[:, :, ms], x_r[:, :, ms])

            q4c = [big.tile([128, 512], bf16, tag=f"q4_{i}", name=f"q4_{i}")
                   for i in range(MC)]
            k4c = [big.tile([128, 512], bf16, tag=f"k4_{i}", name=f"k4_{i}")
                   for i in range(MC)]
            vTc = [big.tile([128, 4, C], bf16, tag=f"vT_{i}", name=f"vT_{i}")
                   for i in range(MC)]
            def st_group(mc, g):
                """Emit the 4 row-tiled S^T matmuls + whole-tile exp."""
                st = ps_st.tile([128, 2048], f32, tag="st", name=f"st_{mc}_{g}")
                for j in range(4):
                    nb = 4 * g + j
                    nc.tensor.matmul(st[:, ds(512 * j, 512)],
                                     k4c[nb // 4][32 * j:32 * (j + 1),
                                                  ds(128 * (nb % 4), 128)],
                                     q4c[mc][32 * j:32 * (j + 1), :],
                                     start=True, stop=True,
                                     tile_position=(32 * j, 0))
                pt = ptp.tile([128, 2048], bf16, tag="pt", name=f"pt_{mc}_{g}")
                nc.scalar.activation(pt[:], st[:], Exp)
                return pt

            # chunk-0 S^T+exp groups are emitted interleaved with the
            # projection chunks they depend on, so the exp pipeline fills
            # during the projection phase's chain-wait bubbles instead of
            # queueing behind all projection matmuls
            PIPE = 6
            ptq = []
            for mc in range(MC):
                ms = ds(512 * mc, 512)
                # q/k: replicated-column weights yield all 4 replicas at
                # once. Their PSUM tiles borrow the sacc/srep banks (idle
                # until attention), so the projection phase runs three
                # independent alloc->read chains instead of one long one.
                for w_t, b4, dst, ptag in ((wqT4, bq4, q4c[mc], "sacc"),
                                           (wkT4, bk4, k4c[mc], "srep")):
                    pp = ps_misc.tile([128, 512], f32, tag=ptag)
                    for o in range(2):
                        nc.tensor.matmul(pp[:], w_t[:, o, :], xs[:, o, ms],
                                         start=(o == 0), stop=(o == 1))
                    # bias-add on DVE, keeping ScalarE free to run the
                    # prefilled exps back-to-back during the proj phase
                    nc.vector.tensor_scalar_add(dst[:], pp[:], b4[:])
                # vT for the 4 key-blocks in this chunk; two blocks share
                # one PSUM tile so the proj phase only cycles the ring twice
                # per chunk (halves the alloc->copy chain that paces it)
                for pair in range(2):
                    pv = ps_out.tile([128, 2, C], f32, tag="out")
                    for bb in range(2):
                        nb = 4 * mc + 2 * pair + bb
                        for o in range(2):
                            nc.tensor.matmul(pv[:, bb, :],
                                             xs[:, o, ds(128 * nb, 128)],
                                             wvT[:, o, :],
                                             start=(o == 0), stop=(o == 1))
                    nc.vector.tensor_copy(
                        vTc[mc][:, ds(2 * pair, 2), :], pv[:])
                # residual base for this chunk: xs += gamma*bv
                for o in range(2):
                    nc.vector.tensor_scalar_add(xs[:, o, ms], xs[:, o, ms],
                                                gbv[:, o:o + 1])
                if mc < PIPE:
                    ptq.append(st_group(0, mc))

            # ---------- main attention loop over query chunks ----------
            # Per group of 4 key-blocks: 4 row-tiled S^T matmuls into one
            # 4-bank PSUM tile, one whole-tile exp on ACT, then (pipelined)
            # 4 adjacent col-tiled s-sums + 8 out accumulations. S^T of group
            # g+1 is emitted before the out-block of g so the PE never waits
            # on ACT in steady state.
            pending_tail = None
            for mc in range(MC):
                ms = ds(512 * mc, 512)
                out_ps = [ps_out.tile([128, 512], f32, tag="out",
                                      name=f"out_{mc}_{cc}")
                          for cc in range(2)]
                s_ps = ps_misc.tile([128, 512], f32, tag="sacc",
                                    name=f"sacc_{mc}")
                for ng in range(NG):
                    gn = mc * NG + ng + PIPE
                    if gn < MC * NG:
                        ptq.append(st_group(gn // NG, gn % NG))
                    pt = ptq.pop(0)
                    for j in range(4):
                        nb = 4 * ng + j
                        for cc in range(2):
                            nc.tensor.matmul(out_ps[cc][:],
                                             vTc[nb // 4][:, nb % 4,
                                                          ds(128 * cc, 128)],
                                             pt[:, ds(512 * j, 512)],
                                             start=(ng == 0 and j == 0),
                                             stop=(ng == NG - 1 and j == 3))
                    # 4 col-tiled partition-sum matmuls, back-to-back
                    for j in range(4):
                        nc.tensor.matmul(s_ps[32 * j:32 * j + 1, :], ones1[:],
                                         pt[:, ds(512 * j, 512)],
                                         start=(ng == 0), stop=(ng == NG - 1),
                                         tile_position=(0, 32 * j))
                    # previous chunk's normalize/output tail: emitted one
                    # group in, so its srep matmul never head-of-line blocks
                    # the queue (its s4 gather DMA is long since done)
                    if ng == 1 and pending_tail is not None:
                        pending_tail()
                        pending_tail = None
                # free the out banks right away; finals run from SBUF copies
                out_sb = []
                for cc in range(2):
                    ob = work.tile([128, 512], f32, tag=f"ob{cc}",
                                   name=f"ob_{mc}_{cc}")
                    nc.vector.tensor_copy(ob[:], out_ps[cc][:])
                    out_sb.append(ob)
                s4c = work.tile([128, 512], f32r, tag="s4c", name=f"s4c_{mc}")
                nc.vector.tensor_copy(s4c[:], s_ps[:])
                # gather the 4 partial rows now (gpsimd queue), so the tail's
                # srep LDWEIGHTS never waits on this DMA at the queue head
                s4_sb = work.tile([4, 512], f32r, tag="s4", name=f"s4_{mc}")
                nc.gpsimd.dma_start(s4_sb[:], s4c[0:97:32, :])

                def tail(mc=mc, ms=ms, out_sb=out_sb, s4_sb=s4_sb):
                    # s: reduce the 4 partial rows + broadcast to 128 rows
                    srep_ps = ps_misc.tile([128, 512], f32, tag="srep")
                    nc.tensor.matmul(srep_ps[:], ones4[:], s4_sb[:],
                                     start=True, stop=True)
                    r_rep = work.tile([128, 512], f32, tag="rrep")
                    nc.vector.reciprocal_approx_fast(r_rep[:], srep_ps[:])
                    nc.vector.tensor_scalar_mul(r_rep[:], r_rep[:], g_col[:])
                    for cc in range(2):
                        y_sb = work.tile([128, 512], f32, tag="y")
                        for h in range(2):
                            hs = ds(256 * h, 256)
                            ys = ds(512 * mc + 256 * h, 256)
                            t_sb = work.tile([128, 256], f32, tag="t")
                            nc.vector.tensor_mul(t_sb[:], out_sb[cc][:, hs],
                                                 r_rep[:, hs])
                            nc.vector.tensor_add(y_sb[:, hs], t_sb[:],
                                                 xs[:, cc, ys])
                            nc.sync.dma_start(y_r[:, cc, ys], y_sb[:, hs])

                pending_tail = tail
            pending_tail()

    nc.compile()
    return nc


def kernel(x, Wq, bq, Wk, bk, Wv, bv, gamma):
    from concourse import bass_utils

    if "nc" not in _NC_CACHE:
        _NC_CACHE["nc"] = _build_nc()
    nc = _NC_CACHE["nc"]

    x = np.ascontiguousarray(np.asarray(x, dtype=np.float32))
    shared = {
        "Wq": np.ascontiguousarray(np.asarray(Wq, dtype=np.float32)),
        "bq": np.ascontiguousarray(np.asarray(bq, dtype=np.float32)),
        "Wk": np.ascontiguousarray(np.asarray(Wk, dtype=np.float32)),
        "bk": np.ascontiguousarray(np.asarray(bk, dtype=np.float32)),
        "Wv": np.ascontiguousarray(np.asarray(Wv, dtype=np.float32)),
        "bv": np.ascontiguousarray(np.asarray(bv, dtype=np.float32)),
        "gamma": np.ascontiguousarray(np.asarray(gamma, dtype=np.float32)),
    }
    in_maps = [dict(shared, x=np.ascontiguousarray(x[i].reshape(C, N)))
               for i in range(B)]

    res = bass_utils.run_bass_kernel_spmd(nc, in_maps, core_ids=list(range(B)))
    y = np.stack([res.results[i]["y"] for i in range(B)], axis=0)
    return y.reshape(B, C, H, W).astype(np.float32)


if __name__ == "__main__":
    rng = np.random.default_rng(0)
    ins = {
        "x": rng.standard_normal((B, C, H, W), dtype=np.float32),
        "Wq": rng.standard_normal((CK, C), dtype=np.float32) / 16,
        "bq": rng.standard_normal((CK,), dtype=np.float32) * 0.01,
        "Wk": rng.standard_normal((CK, C), dtype=np.float32) / 16,
        "bk": rng.standard_normal((CK,), dtype=np.float32) * 0.01,
        "Wv": rng.standard_normal((C, C), dtype=np.float32) / 16,
        "bv": rng.standard_normal((C,), dtype=np.float32) * 0.01,
        "gamma": rng.standard_normal((1,), dtype=np.float32) * 0.1,
    }
    y = kernel(**ins)
    print("kernel output", y.shape, y.dtype)
